# revision 60
# baseline (speedup 1.0000x reference)
"""BlockSparseLinear kernel for Trainium2 (8 NeuronCores, Bass/Tile).

Computes y = x @ W.T + bias with x [8192, 4096] fp32, W [4096, 4096] fp32
(block-masked; treated densely — the 16x16 block granularity is finer than
the PE's 128-deep contraction and the pattern is unstructured, so dense
matmul is the compute roofline), bias [4096].

Numerics: x and W are cast to bf16 on the host (exact rel err vs fp32
reference measured at 2.3e-3, well inside the 2e-2 gate). bf16 matmuls
run 1 cycle/row on the PE (measured 215-216ns per 128x128x512 matmul =
~2.37 GHz sustained) vs fp32r's 227ns, and halve x/W DMA traffic.
PSUM accumulation and the bias epilogue stay fp32.

Sharding: 8-way data-parallel over tokens. Each core computes
yT_c = W @ xT_c + bias for its 1024-token slice.

Per-core kernel (yT layout, outputs on PSUM partitions):
  out[oi=128, t=512] += wT_tile[k=128, oi=128].T @ xT_tile[k=128, t=512]
  - x shard (8.4 MB bf16) resident in SBUF; W streamed column-by-column.
  - bias fused into the PSUM->SBUF eviction on VectorE.
  - x loads issue on the Scalar (Activation) HWDGE queue, w/bias/out on
    the Sync queue: two queues in parallel shorten the critical path to
    the first matmul and keep the PE fed during the DVFS ramp.
  - last output column runs its two t-halves serially so the first
    half's eviction+store hides under the second half's matmuls.

Host side packs inputs so every DMA is contiguous per partition:
  xt[c, p, ko, t] = x[c*1024+t, ko*128+p]          (bf16)
  wp[oc, p, ko, oi] = W[oc*128+oi, ko*128+p]       (bf16, = W.T tiles)
  bs[p, oc] = bias[oc*128+p]                       (fp32)
  output yt[oc, p, t] = y[c*1024+t, oc*128+p]      (fp32)
"""

import os

import numpy as np

N_CORES = 8
TOK = 8192
T_PER_CORE = TOK // N_CORES  # 1024
D_IN = 4096
D_OUT = 4096
P = 128
KO = D_IN // P  # 32 contraction tiles
OC = D_OUT // P  # 32 output column tiles
T_FREE = 512  # moving free dim per matmul
NT = T_PER_CORE // T_FREE  # 2

LAST_EXEC_NS = None

_cache = {}


def _build_bass():
    import concourse.bacc as bacc
    import concourse.mybir as mybir
    import concourse.tile as tile

    f32 = mybir.dt.float32
    bf16 = mybir.dt.bfloat16

    nc = bacc.Bacc(
        "TRN2",
        target_bir_lowering=False,
        debug=False,
        num_devices=N_CORES,
        name="block_sparse_linear",
        dynamic_dma_scratch_size=4096,
    )

    WAVE = 4  # leading output columns processed ko-interleaved during x load

    xt = nc.dram_tensor("xt", [P, KO, T_PER_CORE], bf16, kind="ExternalInput")
    wp = nc.dram_tensor("wp", [OC, P, KO, P], bf16, kind="ExternalInput")
    # Host-repacked copy of the wave head (ko 0..7 of the first WAVE
    # columns), laid out so each head DMA is fully contiguous per
    # partition: wh[p, chunk] with chunk order (c, ko, oi) for ko ranges
    # [0:2), [2:4), [4:8). Strided per-(p,c) 512B descriptor runs made
    # the in-place head DMAs complete ~3us late; contiguous 2-4KB runs
    # land in ~1.5us.
    wh = nc.dram_tensor("wh", [P, WAVE * 8 * P], bf16, kind="ExternalInput")
    bs = nc.dram_tensor("bs", [P, OC], f32, kind="ExternalInput")
    yt = nc.dram_tensor("yt", [OC, P, T_PER_CORE], f32, kind="ExternalOutput")

    with tile.TileContext(nc) as tc:
        with (
            tc.tile_pool(name="xpool", bufs=1) as xpool,
            tc.tile_pool(name="wpool", bufs=WAVE + 2) as wpool,
            tc.tile_pool(name="opool", bufs=4) as opool,
            tc.tile_pool(name="bpool", bufs=1) as bpool,
            tc.tile_pool(name="pspool", bufs=4, space="PSUM") as pspool,
        ):
            # Resident x shard; per (ko, t-half) pieces so ramp matmuls can
            # start as soon as each 128KB piece lands. x rides the Scalar
            # HWDGE queue, w rides Sync: the two streams never serialize
            # behind each other at issue time.
            x_sb = xpool.tile([P, KO, T_PER_CORE], bf16)
            w_wave = [
                wpool.tile([P, KO, P], bf16, tag="w", name=f"w_{oc}")
                for oc in range(WAVE)
            ]
            # Head of the wave's w (ko 0..7 for all WAVE columns) rides in
            # 3 combined contiguous DMAs instead of 4x3 per-column ones:
            # the HWDGE queue serializes DMAs at ~1.4us each regardless of
            # size, so fewer-but-bigger units get column 3's early kos in
            # place before the PE reaches them. Flat [P, n] tiles; the
            # matmul slices the stationary [128,128] block out by offset.
            w_heads = [
                (k0, k1, wpool.tile([P, WAVE * (k1 - k0) * P], bf16, tag="whead", name=f"w_head_{k0}"))
                for k0, k1 in ((0, 2), (2, 4), (4, 8))
            ]

            def dma_x(ko, t):
                nc.scalar.dma_start(
                    x_sb[:, ko, t * T_FREE : (t + 1) * T_FREE],
                    xt[:, ko, t * T_FREE : (t + 1) * T_FREE],
                )

            def dma_w(w_sb, oc, k0, k1):
                nc.sync.dma_start(w_sb[:, k0:k1, :], wp[oc, :, k0:k1, :])

            # Critical path first: the (ko0,t0) matmul needs x(ko0,t0) and
            # the wave-w head — issue those immediately, smallest first,
            # on different queues (x on Scalar's HWDGE queue, w on Sync's,
            # so the streams never serialize behind each other in a FIFO).
            def dma_w_head(i):
                k0, k1, t = w_heads[i]
                off = WAVE * k0 * P
                nc.sync.dma_start(t[:], wh[:, off : off + WAVE * (k1 - k0) * P])

            dma_w_head(0)  # 256KB: ko 0-1 for all 4 wave columns
            dma_x(0, 0)  # 128KB, parallel queue
            # x(0,1) rides sync (2nd in its FIFO, lands ~12us) so the
            # scalar FIFO's ~1.4us serial latency doesn't stack three
            # singles deep before the PE needs ko1.
            nc.sync.dma_start(
                x_sb[:, 0, T_FREE:], xt[:, 0, T_FREE:]
            )
            # ko1 rides as one 256KB unit: the scalar FIFO's serial
            # latency dominates, so two singles would land ko1-t1 ~1.4us
            # later than one combined transfer does.
            nc.scalar.dma_start(x_sb[:, 1, :], xt[:, 1, :])
            dma_w_head(1)
            dma_w_head(2)  # 1MB: ko 4-7 all columns
            # remaining x as 2-ko 512KB chunks: the HWDGE queue's ~1.4us
            # serial per-DMA latency (not engine bandwidth) is what
            # starves the ramp, so fewer-bigger is better once started.
            for ko in range(2, KO, 2):
                nc.scalar.dma_start(
                    x_sb[:, ko : ko + 2, :], xt[:, ko : ko + 2, :]
                )
            bias_sb = bpool.tile([P, OC], f32)
            # remaining wave w on sync, in 16-ko 512KB units: the queue's
            # ~1.4us serial per-DMA latency dominates, so fewer-bigger
            # units give each column's data more deadline margin.
            for k0 in (8, 24):
                for c in range(WAVE):
                    dma_w(w_wave[c], c, k0, k0 + (16 if k0 == 8 else 8))
            nc.sync.dma_start(bias_sb[:], bs[:])

            def evict(oc, ps_ap, t):
                o_sb = opool.tile([P, T_FREE], f32, tag="o", name=f"o_{oc}_{t}")
                # out = psum + bias[p] on VectorE (free-dim-broadcast bias).
                nc.vector.tensor_tensor(
                    o_sb[:],
                    ps_ap,
                    bias_sb[:, oc : oc + 1].to_broadcast([P, T_FREE]),
                    mybir.AluOpType.add,
                )
                # output stores ride the Scalar queue; the Sync queue
                # carries the dense-phase w stream.
                nc.scalar.dma_start(
                    yt[oc, :, t * T_FREE : (t + 1) * T_FREE], o_sb[:]
                )

            # Ramp phase: first WAVE output columns interleaved by ko, so
            # every arriving x piece enables WAVE matmuls. All PSUM tiles
            # are full-row [P, 1024] (two banks); ramp matmuls target one
            # bank-aligned half at a time.
            ps_wave = [
                pspool.tile([P, T_PER_CORE], f32, tag="ps", name=f"ps_{oc}")
                for oc in range(WAVE)
            ]
            # DVFS warmup: the PE clock ramps 0.65 -> 1.2 -> 2.4 GHz only
            # after ~4.5us of continuous busy time, and an idle gap resets
            # the ramp. Burn dummy matmuls during the DMA lead-in so the
            # real matmuls start at full clock instead of paying ~10us of
            # slow-ramp excess. They accumulate into the PSUM tile whose
            # first real use comes latest (col WAVE-1, t1) and are wiped
            # by its real ko0 start=True reset. Operands come from the
            # runtime-reserved dynamic-DMA scratch (always allocated,
            # contents irrelevant — PE timing is data-independent): zero
            # dependencies, so the chain starts the moment the PE queue
            # clears its preamble (~7.5us) instead of waiting on a memset.
            # Overshooting the handoff is cheap (~0.4us); undershooting
            # idles the PE and re-ramps the clock (~2.5us). Warmup start
            # jitters 6.8-8.0us and x(0,0) lands ~12.4us, so 12 blocks
            # (~5.3us of chain) cover the worst case.
            scratch = nc.dma_scratch[:, :2048].bitcast(bf16)
            for _ in range(12):
                nc.tensor.matmul(
                    ps_wave[WAVE - 1][:, T_FREE:],
                    scratch[:, :P],
                    scratch[:, P : P + T_FREE],
                    start=True,
                    stop=True,
                    skip_group_check=True,
                )
            for ko in range(KO):
                for t in range(NT):
                    for oc in range(WAVE):
                        if ko < 2:
                            o = (oc * 2 + ko) * P
                            w_src = w_heads[0][2][:, o : o + P]
                        elif ko < 4:
                            o = (oc * 2 + (ko - 2)) * P
                            w_src = w_heads[1][2][:, o : o + P]
                        elif ko < 8:
                            o = (oc * 4 + (ko - 4)) * P
                            w_src = w_heads[2][2][:, o : o + P]
                        else:
                            w_src = w_wave[oc][:, ko, :]
                        nc.tensor.matmul(
                            ps_wave[oc][:, t * T_FREE : (t + 1) * T_FREE],
                            w_src,
                            x_sb[:, ko, t * T_FREE : (t + 1) * T_FREE],
                            start=(ko == 0),
                            stop=(ko == KO - 1),
                        )
            for oc in range(WAVE):
                for t in range(NT):
                    evict(oc, ps_wave[oc][:, t * T_FREE : (t + 1) * T_FREE], t)

            # Dense phase: x resident; stream one w column per output
            # column. Last column runs t-serial so its first half's
            # eviction+store hides under the second half's matmuls.
            for oc in range(WAVE, OC):
                w_sb = wpool.tile([P, KO, P], bf16, tag="w", name=f"w_{oc}")
                for k0 in range(0, KO, 16):
                    dma_w(w_sb, oc, k0, k0 + 16)
                if oc == OC - 1:
                    # Final column: serial narrowing passes (3x256 then
                    # 2x128) so each slice's eviction+store hides under
                    # the next slice's matmuls; only the last 128-wide
                    # slice's epilogue (~0.35us) is exposed at the tail.
                    # Slices alternate between two tiles and between the
                    # two banks within each — adjacent slices never share
                    # a bank, so no start=True waits on an eviction.
                    ps_fin = [
                        pspool.tile([P, T_PER_CORE], f32, tag="ps", name=f"ps_f{i}")
                        for i in range(2)
                    ]
                    slices = [(0, 256), (256, 256), (512, 256), (768, 128), (896, 128)]
                    for si, (t0, qw) in enumerate(slices):
                        bank = (si // 2) % 2
                        ps_q = ps_fin[si % 2][
                            :, bank * T_FREE : bank * T_FREE + qw
                        ]
                        for ko in range(KO):
                            nc.tensor.matmul(
                                ps_q,
                                w_sb[:, ko, :],
                                x_sb[:, ko, t0 : t0 + qw],
                                start=(ko == 0),
                                stop=(ko == KO - 1),
                            )
                        o_sb = opool.tile([P, qw], f32, tag="o", name=f"oq_{si}")
                        nc.vector.tensor_tensor(
                            o_sb[:],
                            ps_q,
                            bias_sb[:, oc : oc + 1].to_broadcast([P, qw]),
                            mybir.AluOpType.add,
                        )
                        nc.scalar.dma_start(yt[oc, :, t0 : t0 + qw], o_sb[:])
                else:
                    # (N=1024 single matmuls fail the ISA's
                    # s3d3_mm_num_elements check — 512 is the hard cap —
                    # so each column runs two N=512 passes into the two
                    # banks of its [P,1024] tile.)
                    ps_b = pspool.tile(
                        [P, T_PER_CORE], f32, tag="ps", name=f"psb_{oc}"
                    )
                    for ko in range(KO):
                        for t in range(NT):
                            nc.tensor.matmul(
                                ps_b[:, t * T_FREE : (t + 1) * T_FREE],
                                w_sb[:, ko, :],
                                x_sb[:, ko, t * T_FREE : (t + 1) * T_FREE],
                                start=(ko == 0),
                                stop=(ko == KO - 1),
                            )
                    for t in range(NT):
                        evict(oc, ps_b[:, t * T_FREE : (t + 1) * T_FREE], t)

    nc.compile()
    return nc


def _pack_inputs(x, weight, bias):
    import ml_dtypes

    bf16 = ml_dtypes.bfloat16
    x = np.ascontiguousarray(x, dtype=np.float32)
    weight = np.ascontiguousarray(weight, dtype=np.float32)
    bias = np.ascontiguousarray(bias, dtype=np.float32)

    # xt[c, p, ko, t] = x[c*T + t, ko*P + p]
    xt = np.ascontiguousarray(
        x.reshape(N_CORES, T_PER_CORE, KO, P).transpose(0, 3, 2, 1).astype(bf16)
    )
    # wp[oc, p, ko, oi] = W[oc*P + oi, ko*P + p]
    wp = np.ascontiguousarray(
        weight.reshape(OC, P, KO, P).transpose(0, 3, 2, 1).astype(bf16)
    )
    # wh[p, (chunk, c, ko, oi)] — wave-head repack, chunks ko [0:2),[2:4),[4:8)
    WAVE = 4
    wh = np.ascontiguousarray(
        np.concatenate(
            [
                wp[0:WAVE, :, k0:k1, :].transpose(1, 0, 2, 3).reshape(P, -1)
                for k0, k1 in ((0, 2), (2, 4), (4, 8))
            ],
            axis=1,
        )
    )
    # bs[p, oc] = bias[oc*P + p]
    bs = np.ascontiguousarray(bias.reshape(OC, P).T)
    return xt, wp, wh, bs


def kernel(x, weight, bias):
    global LAST_EXEC_NS
    from concourse import bass_utils

    if "nc" not in _cache:
        _cache["nc"] = _build_bass()
    nc = _cache["nc"]

    xt, wp, wh, bs = _pack_inputs(x, weight, bias)

    in_maps = [
        {"xt": xt[c], "wp": wp, "wh": wh, "bs": bs} for c in range(N_CORES)
    ]

    trace = bool(int(os.environ.get("BSL_TRACE", "0")))
    res = bass_utils.run_bass_kernel_spmd(
        nc,
        in_maps,
        core_ids=list(range(N_CORES)),
        trace=trace,
    )
    LAST_EXEC_NS = res.exec_time_ns
    _cache["last_res"] = res

    # yt[c][oc, p, t] -> y[c*T + t, oc*P + p]
    out = np.empty((TOK, D_OUT), dtype=np.float32)
    for c in range(N_CORES):
        yt = res.results[c]["yt"]
        out[c * T_PER_CORE : (c + 1) * T_PER_CORE] = (
            yt.transpose(2, 0, 1).reshape(T_PER_CORE, D_OUT)
        )
    return out


# revision 61
# speedup vs baseline: 1.0015x; 1.0015x over previous
"""BlockSparseLinear kernel for Trainium2 (8 NeuronCores, Bass/Tile).

Computes y = x @ W.T + bias with x [8192, 4096] fp32, W [4096, 4096] fp32
(block-masked; treated densely — the 16x16 block granularity is finer than
the PE's 128-deep contraction and the pattern is unstructured, so dense
matmul is the compute roofline), bias [4096].

Numerics: x and W are cast to bf16 on the host (exact rel err vs fp32
reference measured at 2.3e-3, well inside the 2e-2 gate). bf16 matmuls
run 1 cycle/row on the PE (measured 215-216ns per 128x128x512 matmul =
~2.37 GHz sustained) vs fp32r's 227ns, and halve x/W DMA traffic.
PSUM accumulation and the bias epilogue stay fp32.

Sharding: 8-way data-parallel over tokens. Each core computes
yT_c = W @ xT_c + bias for its 1024-token slice.

Per-core kernel (yT layout, outputs on PSUM partitions):
  out[oi=128, t=512] += wT_tile[k=128, oi=128].T @ xT_tile[k=128, t=512]
  - x shard (8.4 MB bf16) resident in SBUF; W streamed column-by-column.
  - bias fused into the PSUM->SBUF eviction on VectorE.
  - x loads issue on the Scalar (Activation) HWDGE queue, w/bias/out on
    the Sync queue: two queues in parallel shorten the critical path to
    the first matmul and keep the PE fed during the DVFS ramp.
  - last output column runs its two t-halves serially so the first
    half's eviction+store hides under the second half's matmuls.

Host side packs inputs so every DMA is contiguous per partition:
  xt[c, p, ko, t] = x[c*1024+t, ko*128+p]          (bf16)
  wp[oc, p, ko, oi] = W[oc*128+oi, ko*128+p]       (bf16, = W.T tiles)
  bs[p, oc] = bias[oc*128+p]                       (fp32)
  output yt[oc, p, t] = y[c*1024+t, oc*128+p]      (fp32)
"""

import os

import numpy as np

N_CORES = 8
TOK = 8192
T_PER_CORE = TOK // N_CORES  # 1024
D_IN = 4096
D_OUT = 4096
P = 128
KO = D_IN // P  # 32 contraction tiles
OC = D_OUT // P  # 32 output column tiles
T_FREE = 512  # moving free dim per matmul
NT = T_PER_CORE // T_FREE  # 2

LAST_EXEC_NS = None

_cache = {}


def _build_bass():
    import concourse.bacc as bacc
    import concourse.mybir as mybir
    import concourse.tile as tile

    f32 = mybir.dt.float32
    bf16 = mybir.dt.bfloat16

    nc = bacc.Bacc(
        "TRN2",
        target_bir_lowering=False,
        debug=False,
        num_devices=N_CORES,
        name="block_sparse_linear",
        dynamic_dma_scratch_size=4096,
    )

    WAVE = 4  # leading output columns processed ko-interleaved during x load

    xt = nc.dram_tensor("xt", [P, KO, T_PER_CORE], bf16, kind="ExternalInput")
    wp = nc.dram_tensor("wp", [OC, P, KO, P], bf16, kind="ExternalInput")
    # Host-repacked copy of the wave head (ko 0..7 of the first WAVE
    # columns), laid out so each head DMA is fully contiguous per
    # partition: wh[p, chunk] with chunk order (c, ko, oi) for ko ranges
    # [0:2), [2:4), [4:8). Strided per-(p,c) 512B descriptor runs made
    # the in-place head DMAs complete ~3us late; contiguous 2-4KB runs
    # land in ~1.5us.
    wh = nc.dram_tensor("wh", [P, WAVE * 8 * P], bf16, kind="ExternalInput")
    bs = nc.dram_tensor("bs", [P, OC], f32, kind="ExternalInput")
    yt = nc.dram_tensor("yt", [OC, P, T_PER_CORE], f32, kind="ExternalOutput")

    with tile.TileContext(nc) as tc:
        with (
            tc.tile_pool(name="xpool", bufs=1) as xpool,
            tc.tile_pool(name="wpool", bufs=WAVE + 2) as wpool,
            tc.tile_pool(name="opool", bufs=4) as opool,
            tc.tile_pool(name="bpool", bufs=1) as bpool,
            tc.tile_pool(name="pspool", bufs=4, space="PSUM") as pspool,
        ):
            # Resident x shard; per (ko, t-half) pieces so ramp matmuls can
            # start as soon as each 128KB piece lands. x rides the Scalar
            # HWDGE queue, w rides Sync: the two streams never serialize
            # behind each other at issue time.
            x_sb = xpool.tile([P, KO, T_PER_CORE], bf16)
            w_wave = [
                wpool.tile([P, KO, P], bf16, tag="w", name=f"w_{oc}")
                for oc in range(WAVE)
            ]
            # Head of the wave's w (ko 0..7 for all WAVE columns) rides in
            # 3 combined contiguous DMAs instead of 4x3 per-column ones:
            # the HWDGE queue serializes DMAs at ~1.4us each regardless of
            # size, so fewer-but-bigger units get column 3's early kos in
            # place before the PE reaches them. Flat [P, n] tiles; the
            # matmul slices the stationary [128,128] block out by offset.
            w_heads = [
                (k0, k1, wpool.tile([P, WAVE * (k1 - k0) * P], bf16, tag="whead", name=f"w_head_{k0}"))
                for k0, k1 in ((0, 2), (2, 4), (4, 8))
            ]

            def dma_x(ko, t):
                nc.scalar.dma_start(
                    x_sb[:, ko, t * T_FREE : (t + 1) * T_FREE],
                    xt[:, ko, t * T_FREE : (t + 1) * T_FREE],
                )

            def dma_w(w_sb, oc, k0, k1):
                nc.sync.dma_start(w_sb[:, k0:k1, :], wp[oc, :, k0:k1, :])

            # Critical path first: the (ko0,t0) matmul needs x(ko0,t0) and
            # the wave-w head — issue those immediately, smallest first,
            # on different queues (x on Scalar's HWDGE queue, w on Sync's,
            # so the streams never serialize behind each other in a FIFO).
            def dma_w_head(i):
                k0, k1, t = w_heads[i]
                off = WAVE * k0 * P
                nc.sync.dma_start(t[:], wh[:, off : off + WAVE * (k1 - k0) * P])

            dma_w_head(0)  # 256KB: ko 0-1 for all 4 wave columns
            dma_x(0, 0)  # 128KB, parallel queue
            # x(0,1) rides sync (2nd in its FIFO, lands ~12us) so the
            # scalar FIFO's ~1.4us serial latency doesn't stack three
            # singles deep before the PE needs ko1.
            nc.sync.dma_start(
                x_sb[:, 0, T_FREE:], xt[:, 0, T_FREE:]
            )
            # ko1 rides as one 256KB unit: the scalar FIFO's serial
            # latency dominates, so two singles would land ko1-t1 ~1.4us
            # later than one combined transfer does.
            nc.scalar.dma_start(x_sb[:, 1, :], xt[:, 1, :])
            dma_w_head(1)
            dma_w_head(2)  # 1MB: ko 4-7 all columns
            # remaining x as 2-ko 512KB chunks: the HWDGE queue's ~1.4us
            # serial per-DMA latency (not engine bandwidth) is what
            # starves the ramp, so fewer-bigger is better once started.
            for ko in range(2, KO, 2):
                nc.scalar.dma_start(
                    x_sb[:, ko : ko + 2, :], xt[:, ko : ko + 2, :]
                )
            bias_sb = bpool.tile([P, OC], f32)
            # remaining wave w on sync: [8:16) in 8-ko units (tight ko8
            # deadline — a 1MB unit would erode column 3's margin from
            # ~6.7us to ~2.3us), then [16:32) in 16-ko units (loose
            # deadlines, and fewer serial queue slots).
            for c in range(WAVE):
                dma_w(w_wave[c], c, 8, 16)
            for c in range(WAVE):
                dma_w(w_wave[c], c, 16, 32)
            nc.sync.dma_start(bias_sb[:], bs[:])

            def evict(oc, ps_ap, t):
                o_sb = opool.tile([P, T_FREE], f32, tag="o", name=f"o_{oc}_{t}")
                # out = psum + bias[p] on VectorE (free-dim-broadcast bias).
                nc.vector.tensor_tensor(
                    o_sb[:],
                    ps_ap,
                    bias_sb[:, oc : oc + 1].to_broadcast([P, T_FREE]),
                    mybir.AluOpType.add,
                )
                # output stores ride the Scalar queue; the Sync queue
                # carries the dense-phase w stream.
                nc.scalar.dma_start(
                    yt[oc, :, t * T_FREE : (t + 1) * T_FREE], o_sb[:]
                )

            # Ramp phase: first WAVE output columns interleaved by ko, so
            # every arriving x piece enables WAVE matmuls. All PSUM tiles
            # are full-row [P, 1024] (two banks); ramp matmuls target one
            # bank-aligned half at a time.
            ps_wave = [
                pspool.tile([P, T_PER_CORE], f32, tag="ps", name=f"ps_{oc}")
                for oc in range(WAVE)
            ]
            # DVFS warmup: the PE clock ramps 0.65 -> 1.2 -> 2.4 GHz only
            # after ~4.5us of continuous busy time, and an idle gap resets
            # the ramp. Burn dummy matmuls during the DMA lead-in so the
            # real matmuls start at full clock instead of paying ~10us of
            # slow-ramp excess. They accumulate into the PSUM tile whose
            # first real use comes latest (col WAVE-1, t1) and are wiped
            # by its real ko0 start=True reset. Operands come from the
            # runtime-reserved dynamic-DMA scratch (always allocated,
            # contents irrelevant — PE timing is data-independent): zero
            # dependencies, so the chain starts the moment the PE queue
            # clears its preamble (~7.5us) instead of waiting on a memset.
            # Overshooting the handoff is cheap (~0.4us); undershooting
            # idles the PE and re-ramps the clock (~2.5us). Warmup start
            # jitters 6.8-8.0us and x(0,0) lands ~12.4us, so 12 blocks
            # (~5.3us of chain) cover the worst case.
            scratch = nc.dma_scratch[:, :2048].bitcast(bf16)
            for _ in range(12):
                nc.tensor.matmul(
                    ps_wave[WAVE - 1][:, T_FREE:],
                    scratch[:, :P],
                    scratch[:, P : P + T_FREE],
                    start=True,
                    stop=True,
                    skip_group_check=True,
                )
            for ko in range(KO):
                for t in range(NT):
                    for oc in range(WAVE):
                        if ko < 2:
                            o = (oc * 2 + ko) * P
                            w_src = w_heads[0][2][:, o : o + P]
                        elif ko < 4:
                            o = (oc * 2 + (ko - 2)) * P
                            w_src = w_heads[1][2][:, o : o + P]
                        elif ko < 8:
                            o = (oc * 4 + (ko - 4)) * P
                            w_src = w_heads[2][2][:, o : o + P]
                        else:
                            w_src = w_wave[oc][:, ko, :]
                        nc.tensor.matmul(
                            ps_wave[oc][:, t * T_FREE : (t + 1) * T_FREE],
                            w_src,
                            x_sb[:, ko, t * T_FREE : (t + 1) * T_FREE],
                            start=(ko == 0),
                            stop=(ko == KO - 1),
                        )
            for oc in range(WAVE):
                for t in range(NT):
                    evict(oc, ps_wave[oc][:, t * T_FREE : (t + 1) * T_FREE], t)

            # Dense phase: x resident; stream one w column per output
            # column. Last column runs t-serial so its first half's
            # eviction+store hides under the second half's matmuls.
            for oc in range(WAVE, OC):
                w_sb = wpool.tile([P, KO, P], bf16, tag="w", name=f"w_{oc}")
                for k0 in range(0, KO, 16):
                    dma_w(w_sb, oc, k0, k0 + 16)
                if oc == OC - 1:
                    # Final column: serial narrowing passes (3x256 then
                    # 2x128) so each slice's eviction+store hides under
                    # the next slice's matmuls; only the last 128-wide
                    # slice's epilogue (~0.35us) is exposed at the tail.
                    # Slices alternate between two tiles and between the
                    # two banks within each — adjacent slices never share
                    # a bank, so no start=True waits on an eviction.
                    ps_fin = [
                        pspool.tile([P, T_PER_CORE], f32, tag="ps", name=f"ps_f{i}")
                        for i in range(2)
                    ]
                    slices = [(0, 256), (256, 256), (512, 256), (768, 128), (896, 128)]
                    for si, (t0, qw) in enumerate(slices):
                        bank = (si // 2) % 2
                        ps_q = ps_fin[si % 2][
                            :, bank * T_FREE : bank * T_FREE + qw
                        ]
                        for ko in range(KO):
                            nc.tensor.matmul(
                                ps_q,
                                w_sb[:, ko, :],
                                x_sb[:, ko, t0 : t0 + qw],
                                start=(ko == 0),
                                stop=(ko == KO - 1),
                            )
                        o_sb = opool.tile([P, qw], f32, tag="o", name=f"oq_{si}")
                        nc.vector.tensor_tensor(
                            o_sb[:],
                            ps_q,
                            bias_sb[:, oc : oc + 1].to_broadcast([P, qw]),
                            mybir.AluOpType.add,
                        )
                        nc.scalar.dma_start(yt[oc, :, t0 : t0 + qw], o_sb[:])
                else:
                    # (N=1024 single matmuls fail the ISA's
                    # s3d3_mm_num_elements check — 512 is the hard cap —
                    # so each column runs two N=512 passes into the two
                    # banks of its [P,1024] tile.)
                    ps_b = pspool.tile(
                        [P, T_PER_CORE], f32, tag="ps", name=f"psb_{oc}"
                    )
                    for ko in range(KO):
                        for t in range(NT):
                            nc.tensor.matmul(
                                ps_b[:, t * T_FREE : (t + 1) * T_FREE],
                                w_sb[:, ko, :],
                                x_sb[:, ko, t * T_FREE : (t + 1) * T_FREE],
                                start=(ko == 0),
                                stop=(ko == KO - 1),
                            )
                    for t in range(NT):
                        evict(oc, ps_b[:, t * T_FREE : (t + 1) * T_FREE], t)

    nc.compile()
    return nc


def _pack_inputs(x, weight, bias):
    import ml_dtypes

    bf16 = ml_dtypes.bfloat16
    x = np.ascontiguousarray(x, dtype=np.float32)
    weight = np.ascontiguousarray(weight, dtype=np.float32)
    bias = np.ascontiguousarray(bias, dtype=np.float32)

    # xt[c, p, ko, t] = x[c*T + t, ko*P + p]
    xt = np.ascontiguousarray(
        x.reshape(N_CORES, T_PER_CORE, KO, P).transpose(0, 3, 2, 1).astype(bf16)
    )
    # wp[oc, p, ko, oi] = W[oc*P + oi, ko*P + p]
    wp = np.ascontiguousarray(
        weight.reshape(OC, P, KO, P).transpose(0, 3, 2, 1).astype(bf16)
    )
    # wh[p, (chunk, c, ko, oi)] — wave-head repack, chunks ko [0:2),[2:4),[4:8)
    WAVE = 4
    wh = np.ascontiguousarray(
        np.concatenate(
            [
                wp[0:WAVE, :, k0:k1, :].transpose(1, 0, 2, 3).reshape(P, -1)
                for k0, k1 in ((0, 2), (2, 4), (4, 8))
            ],
            axis=1,
        )
    )
    # bs[p, oc] = bias[oc*P + p]
    bs = np.ascontiguousarray(bias.reshape(OC, P).T)
    return xt, wp, wh, bs


def kernel(x, weight, bias):
    global LAST_EXEC_NS
    from concourse import bass_utils

    if "nc" not in _cache:
        _cache["nc"] = _build_bass()
    nc = _cache["nc"]

    xt, wp, wh, bs = _pack_inputs(x, weight, bias)

    in_maps = [
        {"xt": xt[c], "wp": wp, "wh": wh, "bs": bs} for c in range(N_CORES)
    ]

    trace = bool(int(os.environ.get("BSL_TRACE", "0")))
    res = bass_utils.run_bass_kernel_spmd(
        nc,
        in_maps,
        core_ids=list(range(N_CORES)),
        trace=trace,
    )
    LAST_EXEC_NS = res.exec_time_ns
    _cache["last_res"] = res

    # yt[c][oc, p, t] -> y[c*T + t, oc*P + p]
    out = np.empty((TOK, D_OUT), dtype=np.float32)
    for c in range(N_CORES):
        yt = res.results[c]["yt"]
        out[c * T_PER_CORE : (c + 1) * T_PER_CORE] = (
            yt.transpose(2, 0, 1).reshape(T_PER_CORE, D_OUT)
        )
    return out


# revision 62
# speedup vs baseline: 1.0053x; 1.0038x over previous
"""BlockSparseLinear kernel for Trainium2 (8 NeuronCores, Bass/Tile).

Computes y = x @ W.T + bias with x [8192, 4096] fp32, W [4096, 4096] fp32
(block-masked; treated densely — the 16x16 block granularity is finer than
the PE's 128-deep contraction and the pattern is unstructured, so dense
matmul is the compute roofline), bias [4096].

Numerics: x and W are cast to bf16 on the host (exact rel err vs fp32
reference measured at 2.3e-3, well inside the 2e-2 gate). bf16 matmuls
run 1 cycle/row on the PE (measured 215-216ns per 128x128x512 matmul =
~2.37 GHz sustained) vs fp32r's 227ns, and halve x/W DMA traffic.
PSUM accumulation and the bias epilogue stay fp32.

Sharding: 8-way data-parallel over tokens. Each core computes
yT_c = W @ xT_c + bias for its 1024-token slice.

Per-core kernel (yT layout, outputs on PSUM partitions):
  out[oi=128, t=512] += wT_tile[k=128, oi=128].T @ xT_tile[k=128, t=512]
  - x shard (8.4 MB bf16) resident in SBUF; W streamed column-by-column.
  - bias fused into the PSUM->SBUF eviction on VectorE.
  - x loads issue on the Scalar (Activation) HWDGE queue, w/bias/out on
    the Sync queue: two queues in parallel shorten the critical path to
    the first matmul and keep the PE fed during the DVFS ramp.
  - last output column runs its two t-halves serially so the first
    half's eviction+store hides under the second half's matmuls.

Host side packs inputs so every DMA is contiguous per partition:
  xt[c, p, ko, t] = x[c*1024+t, ko*128+p]          (bf16)
  wp[oc, p, ko, oi] = W[oc*128+oi, ko*128+p]       (bf16, = W.T tiles)
  bs[p, oc] = bias[oc*128+p]                       (fp32)
  output yt[oc, p, t] = y[c*1024+t, oc*128+p]      (fp32)
"""

import os

import numpy as np

N_CORES = 8
TOK = 8192
T_PER_CORE = TOK // N_CORES  # 1024
D_IN = 4096
D_OUT = 4096
P = 128
KO = D_IN // P  # 32 contraction tiles
OC = D_OUT // P  # 32 output column tiles
T_FREE = 512  # moving free dim per matmul
NT = T_PER_CORE // T_FREE  # 2

LAST_EXEC_NS = None

_cache = {}


def _build_bass():
    import concourse.bacc as bacc
    import concourse.mybir as mybir
    import concourse.tile as tile

    f32 = mybir.dt.float32
    bf16 = mybir.dt.bfloat16

    nc = bacc.Bacc(
        "TRN2",
        target_bir_lowering=False,
        debug=False,
        num_devices=N_CORES,
        name="block_sparse_linear",
        dynamic_dma_scratch_size=4096,
    )

    WAVE = 4  # leading output columns processed ko-interleaved during x load

    xt = nc.dram_tensor("xt", [P, KO, T_PER_CORE], bf16, kind="ExternalInput")
    wp = nc.dram_tensor("wp", [OC, P, KO, P], bf16, kind="ExternalInput")
    # Host-repacked copy of the wave head (ko 0..7 of the first WAVE
    # columns), laid out so each head DMA is fully contiguous per
    # partition: wh[p, chunk] with chunk order (c, ko, oi) for ko ranges
    # [0:2), [2:4), [4:8). Strided per-(p,c) 512B descriptor runs made
    # the in-place head DMAs complete ~3us late; contiguous 2-4KB runs
    # land in ~1.5us.
    wh = nc.dram_tensor("wh", [P, WAVE * 8 * P], bf16, kind="ExternalInput")
    bs = nc.dram_tensor("bs", [P, OC], f32, kind="ExternalInput")
    yt = nc.dram_tensor("yt", [OC, P, T_PER_CORE], f32, kind="ExternalOutput")

    with tile.TileContext(nc) as tc:
        with (
            tc.tile_pool(name="xpool", bufs=1) as xpool,
            tc.tile_pool(name="wpool", bufs=WAVE + 2) as wpool,
            tc.tile_pool(name="opool", bufs=4) as opool,
            tc.tile_pool(name="bpool", bufs=1) as bpool,
            tc.tile_pool(name="pspool", bufs=4, space="PSUM") as pspool,
        ):
            # Resident x shard; per (ko, t-half) pieces so ramp matmuls can
            # start as soon as each 128KB piece lands. x rides the Scalar
            # HWDGE queue, w rides Sync: the two streams never serialize
            # behind each other at issue time.
            x_sb = xpool.tile([P, KO, T_PER_CORE], bf16)
            w_wave = [
                wpool.tile([P, KO, P], bf16, tag="w", name=f"w_{oc}")
                for oc in range(WAVE)
            ]
            # Head of the wave's w (ko 0..7 for all WAVE columns) rides in
            # 3 combined contiguous DMAs instead of 4x3 per-column ones:
            # the HWDGE queue serializes DMAs at ~1.4us each regardless of
            # size, so fewer-but-bigger units get column 3's early kos in
            # place before the PE reaches them. Flat [P, n] tiles; the
            # matmul slices the stationary [128,128] block out by offset.
            w_heads = [
                (k0, k1, wpool.tile([P, WAVE * (k1 - k0) * P], bf16, tag="whead", name=f"w_head_{k0}"))
                for k0, k1 in ((0, 2), (2, 4), (4, 8))
            ]

            def dma_x(ko, t):
                nc.scalar.dma_start(
                    x_sb[:, ko, t * T_FREE : (t + 1) * T_FREE],
                    xt[:, ko, t * T_FREE : (t + 1) * T_FREE],
                )

            def dma_w(w_sb, oc, k0, k1):
                nc.sync.dma_start(w_sb[:, k0:k1, :], wp[oc, :, k0:k1, :])

            # Critical path first: the (ko0,t0) matmul needs x(ko0,t0) and
            # the wave-w head — issue those immediately, smallest first,
            # on different queues (x on Scalar's HWDGE queue, w on Sync's,
            # so the streams never serialize behind each other in a FIFO).
            def dma_w_head(i):
                k0, k1, t = w_heads[i]
                off = WAVE * k0 * P
                nc.sync.dma_start(t[:], wh[:, off : off + WAVE * (k1 - k0) * P])

            dma_w_head(0)  # 256KB: ko 0-1 for all 4 wave columns
            dma_x(0, 0)  # 128KB, parallel queue
            # x(0,1) rides sync (2nd in its FIFO, lands ~12us) so the
            # scalar FIFO's ~1.4us serial latency doesn't stack three
            # singles deep before the PE needs ko1.
            nc.sync.dma_start(
                x_sb[:, 0, T_FREE:], xt[:, 0, T_FREE:]
            )
            # ko1 rides as one 256KB unit: the scalar FIFO's serial
            # latency dominates, so two singles would land ko1-t1 ~1.4us
            # later than one combined transfer does.
            nc.scalar.dma_start(x_sb[:, 1, :], xt[:, 1, :])
            dma_w_head(1)
            dma_w_head(2)  # 1MB: ko 4-7 all columns
            # remaining x as 2-ko 512KB chunks: the HWDGE queue's ~1.4us
            # serial per-DMA latency (not engine bandwidth) is what
            # starves the ramp, so fewer-bigger is better once started.
            for ko in range(2, KO, 2):
                nc.scalar.dma_start(
                    x_sb[:, ko : ko + 2, :], xt[:, ko : ko + 2, :]
                )
            bias_sb = bpool.tile([P, OC], f32)
            # remaining wave w on sync: [8:16) in 8-ko units (tight ko8
            # deadline — a 1MB unit would erode column 3's margin from
            # ~6.7us to ~2.3us), then [16:32) in 16-ko units (loose
            # deadlines, and fewer serial queue slots).
            for c in range(WAVE):
                dma_w(w_wave[c], c, 8, 16)
            for c in range(WAVE):
                dma_w(w_wave[c], c, 16, 32)
            nc.sync.dma_start(bias_sb[:], bs[:])

            def evict(oc, ps_ap, t):
                o_sb = opool.tile([P, T_FREE], f32, tag="o", name=f"o_{oc}_{t}")
                # out = psum + bias[p] on VectorE (free-dim-broadcast bias).
                nc.vector.tensor_tensor(
                    o_sb[:],
                    ps_ap,
                    bias_sb[:, oc : oc + 1].to_broadcast([P, T_FREE]),
                    mybir.AluOpType.add,
                )
                # output stores ride the Scalar queue; the Sync queue
                # carries the dense-phase w stream.
                nc.scalar.dma_start(
                    yt[oc, :, t * T_FREE : (t + 1) * T_FREE], o_sb[:]
                )

            # Ramp phase: first WAVE output columns interleaved by ko, so
            # every arriving x piece enables WAVE matmuls. All PSUM tiles
            # are full-row [P, 1024] (two banks); ramp matmuls target one
            # bank-aligned half at a time.
            ps_wave = [
                pspool.tile([P, T_PER_CORE], f32, tag="ps", name=f"ps_{oc}")
                for oc in range(WAVE)
            ]
            # DVFS warmup: the PE clock ramps 0.65 -> 1.2 -> 2.4 GHz only
            # after ~4.5us of continuous busy time, and an idle gap resets
            # the ramp. Burn dummy matmuls during the DMA lead-in so the
            # real matmuls start at full clock instead of paying ~10us of
            # slow-ramp excess. They accumulate into the PSUM tile whose
            # first real use comes latest (col WAVE-1, t1) and are wiped
            # by its real ko0 start=True reset. Operands come from the
            # runtime-reserved dynamic-DMA scratch (always allocated,
            # contents irrelevant — PE timing is data-independent): zero
            # dependencies, so the chain starts the moment the PE queue
            # clears its preamble (~7.5us) instead of waiting on a memset.
            # Overshooting the handoff is cheap (~0.4us); undershooting
            # idles the PE and re-ramps the clock (~2.5us). Warmup start
            # jitters 6.8-8.0us and x(0,0) lands ~12.4us, so 12 blocks
            # (~5.3us of chain) cover the worst case.
            scratch = nc.dma_scratch[:, :2048].bitcast(bf16)
            for _ in range(12):
                nc.tensor.matmul(
                    ps_wave[WAVE - 1][:, T_FREE:],
                    scratch[:, :P],
                    scratch[:, P : P + T_FREE],
                    start=True,
                    stop=True,
                    skip_group_check=True,
                )
            for ko in range(KO):
                for t in range(NT):
                    for oc in range(WAVE):
                        if ko < 2:
                            o = (oc * 2 + ko) * P
                            w_src = w_heads[0][2][:, o : o + P]
                        elif ko < 4:
                            o = (oc * 2 + (ko - 2)) * P
                            w_src = w_heads[1][2][:, o : o + P]
                        elif ko < 8:
                            o = (oc * 4 + (ko - 4)) * P
                            w_src = w_heads[2][2][:, o : o + P]
                        else:
                            w_src = w_wave[oc][:, ko, :]
                        nc.tensor.matmul(
                            ps_wave[oc][:, t * T_FREE : (t + 1) * T_FREE],
                            w_src,
                            x_sb[:, ko, t * T_FREE : (t + 1) * T_FREE],
                            start=(ko == 0),
                            stop=(ko == KO - 1),
                        )
            for oc in range(WAVE):
                for t in range(NT):
                    evict(oc, ps_wave[oc][:, t * T_FREE : (t + 1) * T_FREE], t)

            # Dense phase: x resident; stream one w column per output
            # column. Last column runs t-serial so its first half's
            # eviction+store hides under the second half's matmuls.
            for oc in range(WAVE, OC):
                w_sb = wpool.tile([P, KO, P], bf16, tag="w", name=f"w_{oc}")
                for k0 in range(0, KO, 8):
                    dma_w(w_sb, oc, k0, k0 + 8)
                if oc == OC - 1:
                    # Final column: serial narrowing passes (3x256 then
                    # 2x128) so each slice's eviction+store hides under
                    # the next slice's matmuls; only the last 128-wide
                    # slice's epilogue (~0.35us) is exposed at the tail.
                    # Slices alternate between two tiles and between the
                    # two banks within each — adjacent slices never share
                    # a bank, so no start=True waits on an eviction.
                    ps_fin = [
                        pspool.tile([P, T_PER_CORE], f32, tag="ps", name=f"ps_f{i}")
                        for i in range(2)
                    ]
                    slices = [(0, 256), (256, 256), (512, 256), (768, 128), (896, 128)]
                    for si, (t0, qw) in enumerate(slices):
                        bank = (si // 2) % 2
                        ps_q = ps_fin[si % 2][
                            :, bank * T_FREE : bank * T_FREE + qw
                        ]
                        for ko in range(KO):
                            nc.tensor.matmul(
                                ps_q,
                                w_sb[:, ko, :],
                                x_sb[:, ko, t0 : t0 + qw],
                                start=(ko == 0),
                                stop=(ko == KO - 1),
                            )
                        o_sb = opool.tile([P, qw], f32, tag="o", name=f"oq_{si}")
                        nc.vector.tensor_tensor(
                            o_sb[:],
                            ps_q,
                            bias_sb[:, oc : oc + 1].to_broadcast([P, qw]),
                            mybir.AluOpType.add,
                        )
                        nc.scalar.dma_start(yt[oc, :, t0 : t0 + qw], o_sb[:])
                else:
                    # (N=1024 single matmuls fail the ISA's
                    # s3d3_mm_num_elements check — 512 is the hard cap —
                    # so each column runs two N=512 passes into the two
                    # banks of its [P,1024] tile.)
                    ps_b = pspool.tile(
                        [P, T_PER_CORE], f32, tag="ps", name=f"psb_{oc}"
                    )
                    for ko in range(KO):
                        for t in range(NT):
                            nc.tensor.matmul(
                                ps_b[:, t * T_FREE : (t + 1) * T_FREE],
                                w_sb[:, ko, :],
                                x_sb[:, ko, t * T_FREE : (t + 1) * T_FREE],
                                start=(ko == 0),
                                stop=(ko == KO - 1),
                            )
                    for t in range(NT):
                        evict(oc, ps_b[:, t * T_FREE : (t + 1) * T_FREE], t)

    nc.compile()
    return nc


def _pack_inputs(x, weight, bias):
    import ml_dtypes

    bf16 = ml_dtypes.bfloat16
    x = np.ascontiguousarray(x, dtype=np.float32)
    weight = np.ascontiguousarray(weight, dtype=np.float32)
    bias = np.ascontiguousarray(bias, dtype=np.float32)

    # xt[c, p, ko, t] = x[c*T + t, ko*P + p]
    xt = np.ascontiguousarray(
        x.reshape(N_CORES, T_PER_CORE, KO, P).transpose(0, 3, 2, 1).astype(bf16)
    )
    # wp[oc, p, ko, oi] = W[oc*P + oi, ko*P + p]
    wp = np.ascontiguousarray(
        weight.reshape(OC, P, KO, P).transpose(0, 3, 2, 1).astype(bf16)
    )
    # wh[p, (chunk, c, ko, oi)] — wave-head repack, chunks ko [0:2),[2:4),[4:8)
    WAVE = 4
    wh = np.ascontiguousarray(
        np.concatenate(
            [
                wp[0:WAVE, :, k0:k1, :].transpose(1, 0, 2, 3).reshape(P, -1)
                for k0, k1 in ((0, 2), (2, 4), (4, 8))
            ],
            axis=1,
        )
    )
    # bs[p, oc] = bias[oc*P + p]
    bs = np.ascontiguousarray(bias.reshape(OC, P).T)
    return xt, wp, wh, bs


def kernel(x, weight, bias):
    global LAST_EXEC_NS
    from concourse import bass_utils

    if "nc" not in _cache:
        _cache["nc"] = _build_bass()
    nc = _cache["nc"]

    xt, wp, wh, bs = _pack_inputs(x, weight, bias)

    in_maps = [
        {"xt": xt[c], "wp": wp, "wh": wh, "bs": bs} for c in range(N_CORES)
    ]

    trace = bool(int(os.environ.get("BSL_TRACE", "0")))
    res = bass_utils.run_bass_kernel_spmd(
        nc,
        in_maps,
        core_ids=list(range(N_CORES)),
        trace=trace,
    )
    LAST_EXEC_NS = res.exec_time_ns
    _cache["last_res"] = res

    # yt[c][oc, p, t] -> y[c*T + t, oc*P + p]
    out = np.empty((TOK, D_OUT), dtype=np.float32)
    for c in range(N_CORES):
        yt = res.results[c]["yt"]
        out[c * T_PER_CORE : (c + 1) * T_PER_CORE] = (
            yt.transpose(2, 0, 1).reshape(T_PER_CORE, D_OUT)
        )
    return out
